# revision 1
# baseline (speedup 1.0000x reference)
"""Trainium2 Bass kernel for nn_CSCLoss: multi-scale bilinear point-sampling
cosine-consistency loss.

loss = 1 - mean_{pairs,(b,n)} <normalize(sample(feat_i, p_bn)), normalize(sample(feat_j, p_bn))>

Sharding: data-parallel over batch — 32 images -> 8 cores x 4 images; the
host sums the 8 per-core partial sums and applies the 1 - total/count
epilogue (the all-reduce of the sharding hint, done on 8 scalars).

Per-core dataflow (dense, HBM-bandwidth-bound):
 - All per-point scalar math (pixel coords, floor, lerp weights, gather
   indices) runs on partition 0 in [1,128]-wide vector ops from `boxes`.
 - Gather indices are laid out in ap_gather's wrapped format and replicated
   to all 8 DVE 16-partition groups with a 0-stride DRAM->SBUF DMA; bilinear
   weights are replicated to all 128 partitions the same way.
 - Feature maps stream through SBUF as multi-image [128ch, nb*H*W] tiles
   (21 MiB/core at DMA line rate — the roofline) split over two HWDGE rings;
   gpsimd.ap_gather (batched — each dispatch has ~4us fixed cost) extracts
   the 4 bilinear corners per point, DVE applies the lerp weights and
   reduces to sampled vectors v[c, col], col = b*32 + s*4 + rb.
 - Channel reductions (squared norms, pairwise dots) are ones-vector
   matmuls on PE accumulating the two 128-channel chunks into PSUM [1,128].
 - The cosine epilogue runs on partition 0 and emits one [1,1] partial.
"""

import sys
from contextlib import ExitStack

import numpy as np

if "/opt/trn_rl_repo" not in sys.path:
    sys.path.insert(0, "/opt/trn_rl_repo")

B, N, C = 32, 32, 256
LEVELS = [(64, 64), (32, 32), (16, 16)]  # (H, W)
NB = [1, 1, 1]                           # images per gather batch
LORDER = [2, 1, 0]                       # small levels first (early Pool start)
N_CORES = 8
BL = B // N_CORES          # images per core
NPTS = BL * N              # 128 points per core
PAIRS = [(0, 1), (0, 2), (1, 2)]
EPS = 1e-12

_CACHE = {}


def _build_program():
    from concourse import bacc, bass, mybir, tile, library_config

    dt = mybir.dt
    AL = mybir.AluOpType

    nc = bacc.Bacc("TRN2", target_bir_lowering=False, debug=False)

    feats = [
        nc.dram_tensor(f"feat{i}", [BL, C, H, W], dt.float32, kind="ExternalInput")
        for i, (H, W) in enumerate(LEVELS)
    ]
    boxes = nc.dram_tensor("boxes", [BL, N, 4], dt.float32, kind="ExternalInput")
    out = nc.dram_tensor("out", [1, 1], dt.float32, kind="ExternalOutput")

    with tile.TileContext(nc) as tc, ExitStack() as ctx:
        pool = ctx.enter_context(tc.tile_pool(name="sbuf", bufs=1))
        pa = ctx.enter_context(tc.tile_pool(name="pa", bufs=1))
        pstream = ctx.enter_context(tc.tile_pool(name="stream", bufs=1))
        pwork = ctx.enter_context(tc.tile_pool(name="work", bufs=2))
        ppsum = ctx.enter_context(tc.tile_pool(name="psum", bufs=1, space="PSUM"))
        pdram = ctx.enter_context(tc.tile_pool(name="dram", bufs=1, space="DRAM"))

        nc.gpsimd.load_library(library_config.ap_gather)

        # constants for PE-based broadcasts
        ones1 = pool.tile([1, 128], dt.float32)
        nc.vector.memset(ones1[:], 1.0)
        # REPLf[k, q] = 1.0 iff q % 16 == k  (block-replicate [16,*] -> [128,*])
        repl_i = pool.tile([16, 128], dt.int32)
        nc.gpsimd.iota(repl_i[:], pattern=[[1, 128]], base=0, channel_multiplier=15)
        nc.vector.tensor_scalar(
            out=repl_i[:], in0=repl_i[:], scalar1=15, scalar2=None,
            op0=AL.bitwise_and,
        )
        replf = pool.tile([16, 128], dt.float32)
        nc.vector.tensor_scalar(
            out=replf[:], in0=repl_i[:], scalar1=0, scalar2=None, op0=AL.is_equal,
        )

        # ---- boxes load first on the scalar ring (phase A needs it) ----
        bxr = pool.tile([1, BL * N * 4], dt.float32)  # [1, 512] flat boxes
        nc.scalar.dma_start(
            out=bxr[:].rearrange("o (a f) -> o a f", a=BL),
            in_=boxes.rearrange("b n c -> b (n c)"),
        )

        # ---- feature-map streaming DMAs, issued up front ----
        # small levels first on the scalar ring (their gathers start the Pool
        # pipeline early); lvl0 on the sync ring.
        dma_eng = [nc.sync, nc.scalar, nc.scalar]
        T_tiles = {}
        for li in LORDER:
            H, W = LEVELS[li]
            HW = H * W
            nb = NB[li]
            fview = feats[li].rearrange("b c h w -> c b (h w)")
            SBUFS = [5, 8, 8]
            for u in range(BL // nb):
                for ch in range(2):
                    T = pstream.tile(
                        [128, nb * HW], dt.float32, name=f"T{li}_{u}_{ch}",
                        tag=f"T{li}", bufs=SBUFS[li],
                    )
                    dma_eng[li].dma_start(
                        out=T[:].rearrange("c (b q) -> c b q", b=nb),
                        in_=fview[ch * 128:(ch + 1) * 128, u * nb:(u + 1) * nb, :],
                    )
                    T_tiles[(li, u, ch)] = T

        # ---- Phase A: per-point scalar math on partition 0 (DVE) ----
        bxv = bxr[:].rearrange("o (j c) -> o j c", c=4)
        cx = bxv[:, :, 0]  # [1, 128] stride 4
        cy = bxv[:, :, 1]

        def axis_prep(coord, E, ax):
            """pixel coord p=clip(c*(E-1),0,E-1); e0=clamp(floor(p),0,E-2);
            w=p-e0. floor via 16.16 fixed point (exact *2^16; conversion
            error <=2^-16 absorbed by the lerp weight)."""
            pf = pa.tile([1, NPTS], dt.float32, name=f"pf{ax}", tag=f"pf{ax}")
            nc.vector.tensor_scalar(
                out=pf[:], in0=coord, scalar1=float(E - 1), scalar2=0.0,
                op0=AL.mult, op1=AL.max,
            )
            nc.vector.tensor_scalar_min(out=pf[:], in0=pf[:], scalar1=float(E - 1))
            pxs = pa.tile([1, NPTS], dt.float32, name=f"pxs{ax}", tag=f"pxs{ax}")
            nc.vector.tensor_scalar(
                out=pxs[:], in0=pf[:], scalar1=65536.0, scalar2=None, op0=AL.mult,
            )
            ifx = pa.tile([1, NPTS], dt.int32, name=f"ifx{ax}", tag=f"ifx{ax}")
            nc.vector.tensor_copy(out=ifx[:], in_=pxs[:])
            x0i = pa.tile([1, NPTS], dt.int32, name=f"x0i{ax}", tag=f"x0i{ax}")
            nc.vector.tensor_scalar(
                out=x0i[:], in0=ifx[:], scalar1=16, scalar2=None,
                op0=AL.arith_shift_right,
            )
            e0 = pa.tile([1, NPTS], dt.float32, name=f"e0{ax}", tag=f"e0{ax}")
            nc.vector.tensor_copy(out=e0[:], in_=x0i[:])
            nc.vector.tensor_scalar_min(out=e0[:], in0=e0[:], scalar1=float(E - 2))
            we = pa.tile([1, NPTS], dt.float32, name=f"we{ax}", tag=f"we{ax}")
            nc.vector.tensor_tensor(out=we[:], in0=pf[:], in1=e0[:], op=AL.subtract)
            return e0, we

        V = [
            [pool.tile([128, NPTS], dt.float32, name=f"V{li}_{ch}") for ch in range(2)]
            for li in range(3)
        ]
        for li in LORDER:
            H, W = LEVELS[li]
            HW = H * W
            nb = NB[li]
            x0f, wx = axis_prep(cx, W, "x")
            y0f, wy = axis_prep(cy, H, "y")

            # basefu[point(b,n)] = y0*W + x0 + (b % nb)*HW  (unit-local image
            # offset folded in; values < nb*HW <= 16384 fit int16)
            basef = pa.tile([1, NPTS], dt.float32, name="basef", tag="basef")
            nc.vector.tensor_scalar(
                out=basef[:], in0=y0f[:], scalar1=float(W), scalar2=None,
                op0=AL.mult,
            )
            nc.vector.tensor_tensor(
                out=basef[:], in0=basef[:], in1=x0f[:], op=AL.add
            )
            basef_b = basef[:].rearrange("o (b n) -> o b n", b=BL)
            for b in range(BL):
                off = float((b % nb) * HW)
                if off:
                    nc.vector.tensor_scalar(
                        out=basef_b[:, b], in0=basef_b[:, b],
                        scalar1=off, scalar2=None, op0=AL.add,
                    )

            # wrapped index row: flat layout r*32 + b*8 + s, r=rb*4+k,
            # value = basefu[point(b, 4s+rb)] + dk(k), dk = (k//2)*W + k%2
            srow = pa.tile([1, 16 * 32], dt.float32, name="srow", tag="srow")
            srow_v = srow[:].rearrange("o (r b s) -> o r b s", r=16, b=BL)
            basef_v = basef[:].rearrange("o (b s f) -> o b s f", b=BL, f=4)
            for rb in range(4):
                for k in range(4):
                    dk = float((k // 2) * W + (k % 2))
                    nc.vector.tensor_scalar(
                        out=srow_v[:, rb * 4 + k],
                        in0=basef_v[:, :, :, rb],
                        scalar1=dk, scalar2=None, op0=AL.add,
                    )
            sidx = pdram.tile([16, 32], dt.float32, name=f"sidx{li}")
            nc.gpsimd.dma_start(
                out=sidx[:], in_=srow[:].rearrange("o (r c) -> o r c", r=16),
            )
            s16f = pa.tile([16, 32], dt.float32, name="s16f", tag="s16f")
            nc.gpsimd.dma_start(out=s16f[:], in_=sidx[:])
            widx_ps = ppsum.tile([128, 32], dt.float32, name=f"widxps{li}", tag="widxps")
            nc.tensor.matmul(
                widx_ps[:], replf[:], s16f[:], start=True, stop=True,
            )
            widx = pool.tile([128, 32], dt.int16, name=f"widx{li}")
            nc.vector.tensor_copy(out=widx[:], in_=widx_ps[:])

            # corner weights, k = yi*2 + xi, packed k-major then reordered to
            # the gather-output column order (b, s, rb, k)
            w1x = pa.tile([1, NPTS], dt.float32, name="w1x", tag="w1x")
            nc.vector.tensor_scalar(
                out=w1x[:], in0=wx[:], scalar1=-1.0, scalar2=1.0,
                op0=AL.mult, op1=AL.add,
            )
            w1y = pa.tile([1, NPTS], dt.float32, name="w1y", tag="w1y")
            nc.vector.tensor_scalar(
                out=w1y[:], in0=wy[:], scalar1=-1.0, scalar2=1.0,
                op0=AL.mult, op1=AL.add,
            )
            wkt = pa.tile([1, 4 * NPTS], dt.float32, name="wkt", tag="wkt")
            for k, (wyt, wxt) in enumerate(
                [(w1y, w1x), (w1y, wx), (wy, w1x), (wy, wx)]
            ):
                nc.vector.tensor_tensor(
                    out=wkt[:, k * NPTS:(k + 1) * NPTS],
                    in0=wyt[:], in1=wxt[:], op=AL.mult,
                )
            wrow = pa.tile([1, NPTS * 4], dt.float32, name="wrow", tag="wrow")
            # wrow col = b*128 + s*16 + rb*4 + k <- wkt[k*128 + b*32 + s*4 + rb]
            wkt_v = wkt[:].rearrange(
                "o (k b s rb) -> o k b s rb", k=4, b=BL, s=8
            )
            wrow_v = wrow[:].rearrange(
                "o (b s rb k) -> o b s rb k", b=BL, s=8, rb=4
            )
            for b in range(BL):
                nc.vector.tensor_copy(
                    out=wrow_v[:, b],
                    in_=wkt_v[:, :, b].rearrange("o k s rb -> o s rb k"),
                )
            wb_ps = ppsum.tile([128, NPTS * 4], dt.float32, name=f"wbps{li}", tag="wbps")
            nc.tensor.matmul(wb_ps[:], ones1[:], wrow[:], start=True, stop=True)
            wb = pool.tile([128, NPTS * 4], dt.float32, name=f"wb{li}")
            nc.vector.tensor_copy(out=wb[:], in_=wb_ps[:])
            # ---- this level's gathers + lerp (V col = b*32 + s*4 + rb) ----
            ncols = nb * 128
            for u in range(BL // nb):
                for ch in range(2):
                    T = T_tiles[(li, u, ch)]
                    og = pwork.tile(
                        [128, ncols], dt.float32, name=f"og{li}", tag="og"
                    )
                    nc.gpsimd.ap_gather(
                        out_ap=og[:], in_ap=T[:],
                        idxs_ap=widx[:, u * nb * 8:(u + 1) * nb * 8],
                        channels=128, num_elems=nb * HW, d=1, num_idxs=ncols,
                    )
                    nc.vector.tensor_tensor(
                        out=og[:], in0=og[:],
                        in1=wb[:, u * ncols:(u + 1) * ncols], op=AL.mult,
                    )
                    nc.vector.tensor_reduce(
                        out=V[li][ch][:, u * nb * 32:(u + 1) * nb * 32],
                        in_=og[:].rearrange("c (n f) -> c n f", f=4),
                        axis=mybir.AxisListType.X, op=AL.add,
                    )

        # ---- Phase C: channel reductions via ones-matmul into PSUM ----
        ones = pool.tile([128, 1], dt.float32)
        nc.vector.memset(ones[:], 1.0)

        def colsum(name, make_in):
            ps = ppsum.tile([1, NPTS], dt.float32, name=name)
            for ch in range(2):
                prod = pwork.tile(
                    [128, NPTS], dt.float32, name=f"prod{name}{ch}", tag="prod"
                )
                make_in(prod, ch)
                nc.tensor.matmul(
                    ps[:], ones[:], prod[:], start=(ch == 0), stop=(ch == 1),
                )
            sb = pool.tile([1, NPTS], dt.float32, name=f"sb{name}")
            nc.vector.tensor_copy(out=sb[:], in_=ps[:])
            return sb

        ss = [
            colsum(
                f"ss{li}",
                lambda prod, ch, li=li: nc.vector.tensor_tensor(
                    out=prod[:], in0=V[li][ch][:], in1=V[li][ch][:], op=AL.mult
                ),
            )
            for li in range(3)
        ]
        dots = {}
        for i, j in PAIRS:
            dots[(i, j)] = colsum(
                f"d{i}{j}",
                lambda prod, ch, i=i, j=j: nc.vector.tensor_tensor(
                    out=prod[:], in0=V[i][ch][:], in1=V[j][ch][:], op=AL.mult
                ),
            )

        # ---- Phase D: cosine epilogue on partition 0 ----
        rns = []
        for li in range(3):
            nrm = pool.tile([1, NPTS], dt.float32, name=f"nrm{li}")
            nc.scalar.sqrt(out=nrm[:], in_=ss[li][:])
            nc.vector.tensor_scalar_max(out=nrm[:], in0=nrm[:], scalar1=EPS)
            rn = pool.tile([1, NPTS], dt.float32, name=f"rn{li}")
            nc.vector.reciprocal(out=rn[:], in_=nrm[:])
            rns.append(rn)

        tot = pool.tile([1, NPTS], dt.float32)
        first = True
        for i, j in PAIRS:
            t = pool.tile([1, NPTS], dt.float32, name=f"t{i}{j}")
            nc.vector.tensor_tensor(
                out=t[:], in0=dots[(i, j)][:], in1=rns[i][:], op=AL.mult
            )
            nc.vector.tensor_tensor(out=t[:], in0=t[:], in1=rns[j][:], op=AL.mult)
            if first:
                nc.vector.tensor_copy(out=tot[:], in_=t[:])
                first = False
            else:
                nc.vector.tensor_tensor(out=tot[:], in0=tot[:], in1=t[:], op=AL.add)

        res = pool.tile([1, 1], dt.float32)
        nc.vector.tensor_reduce(
            out=res[:], in_=tot[:], axis=mybir.AxisListType.X, op=AL.add
        )
        nc.sync.dma_start(out=out.ap(), in_=res[:])

    nc.compile()
    return nc


def _get_program():
    if "nc" not in _CACHE:
        _CACHE["nc"] = _build_program()
    return _CACHE["nc"]


def _run_device(feat0, feat1, feat2, boxes, **run_kwargs):
    """Shard inputs batch-wise over the 8 cores, run the SPMD program, and
    return the BassKernelResults (one {"out": [1,1]} per core)."""
    from concourse.bass_utils import run_bass_kernel_spmd

    nc = _get_program()

    feats = [
        np.ascontiguousarray(np.asarray(f, dtype=np.float32))
        for f in (feat0, feat1, feat2)
    ]
    boxes = np.ascontiguousarray(np.asarray(boxes, dtype=np.float32))

    in_maps = []
    for k in range(N_CORES):
        sl = slice(k * BL, (k + 1) * BL)
        in_maps.append(
            {
                "feat0": feats[0][sl],
                "feat1": feats[1][sl],
                "feat2": feats[2][sl],
                "boxes": boxes[sl],
            }
        )

    return run_bass_kernel_spmd(
        nc, in_maps, core_ids=list(range(N_CORES)), **run_kwargs
    )


def kernel(feat0, feat1, feat2, boxes):
    r = _run_device(feat0, feat1, feat2, boxes)
    total = np.float64(0.0)
    for m in r.results:
        total += np.float64(m["out"].reshape(-1)[0])

    count = B * N * len(PAIRS)
    avg = np.float32(total) / np.float32(count)
    loss = np.float32(1.0) - avg
    loss = np.nan_to_num(loss, nan=0.0, posinf=1.0, neginf=0.0)
    return np.array(np.clip(loss, 0.0, 2.0), dtype=np.float32)

